# revision 37
# baseline (speedup 1.0000x reference)
"""Bidirectional minGRU (nn_MinGRU2) Trainium2 Bass kernel.

Full input x: [16, 512, 4096] f32. Channel layout per batch:
    0:128    forward h        128:256  forward g
    256:384  backward h       384:512  backward g
Output [16, 256, 4096]: out[:, 0:128] = forward minGRU, out[:, 128:256] =
backward minGRU (scanned right-to-left over L).

The log-space reference reduces to the direct linear recurrence per
(b, channel) lane:
    sig  = sigmoid(g);  coef = sigmoid(-g);  v = h * sig
    y[t] = coef[t] * y[t-1] + v[t]
which maps to one DVE tensor_tensor_scan per [128-lane, L-chunk] tile, with
ACT computing the sigmoids and DVE the multiply. The backward direction
runs the same scan through reversed (negative-stride) access patterns, so
no explicit flip pass is needed.

HBM traffic (the roofline: ~360 GB/s aggregate per core) is minimized by
quantizing the inputs on the host: h is cast to fp16, and g to int8 with a
per-tensor scale (sigmoid's |derivative| <= 1/4 makes a uniform int8 grid on
g ~3x more accurate than fp8; measured end-to-end rel err ~8e-3 vs the 2e-2
gate). The ACT sigmoid applies the dequant scale for free via its
scale operand, so no dequant pass exists on any engine. The output is
stored as fp16 and upcast on the host.

Engine budget per core (4 streams x 4096 cols): DMA 10.5 MB ~= 29 us;
ACT sigmoid pass ~16 us (so only some coef passes fit there); DVE scan is
mode-less (1 col/cycle, 17.1 us) + fp16 2x mult 8.5 us, so the coef
(= 1 - sig) passes are split ACT/DVE per-stream to balance both near the
DMA roofline.

Sharding: fully data-parallel over batch - 16 batches / 8 cores = 2 per
core; every (b, lane) recurrence is independent and L stays contiguous.
"""
import numpy as np

import concourse.bacc as bacc
import concourse.mybir as mybir
import concourse.tile as tile
from concourse.bass_utils import run_bass_kernel_spmd

B, H, L = 16, 512, 4096
N_CORES = 8
B_PC = B // N_CORES  # batches per core

P = 128
F16 = mybir.dt.float16
I8 = mybir.dt.int8
MULT = mybir.AluOpType.mult
ADD = mybir.AluOpType.add
SIGMOID = mybir.ActivationFunctionType.Sigmoid

CHUNK = 2048
BUFS = 3
IN_BUFS = 4
OUT_BUFS = 8

# g-quantization scale baked into the ACT sigmoid's scale operand. The value
# only affects accuracy (kernel() recomputes the actual absmax at runtime and
# rebuilds if it differs); timing is scale-independent.
SG_DEFAULT = 5.2201 / 127.0


def _emit(tc: tile.TileContext, xh, xg, out, sg=SG_DEFAULT, chunk=CHUNK,
          bufs=BUFS, out_bufs=OUT_BUFS, store_eng=2, in_bufs=IN_BUFS, first=0,
          last=512, coef_acts=2, coef_dve_mod=4, mult_pool_mod=0,
          scan_pool_mod=0, scan_pool_k=(), mult_pool_k=(), coef_act_k=(),
          load_split=1, load_eng="sync", scan_split=1,
          scan_split_last=1, store_split=1, store_delay=2, mult_pool=0,
          skip_store=0, skip_scan=0, skip_mult=0, skip_coef=0, skip_sig=0,
          pools=None):
    nc = tc.nc
    # chunk schedule over L; `first` splits a smaller leading chunk off the
    # first full chunk so compute/stores start earlier (shorter pipeline
    # fill); `last` splits a small trailing chunk off the final one so the
    # post-last-load serial tail (sig->mult->scan->store) is short
    sizes = [chunk] * (L // chunk)
    if first:
        sizes = [first, chunk - first] + sizes[1:]
    if last:
        sizes = sizes[:-1] + [sizes[-1] - last, last]
    # streams: (batch, direction); direction 0 = forward, 1 = backward
    streams = [(b, d) for b in range(B_PC) for d in (0, 1)]
    carries = {s: None for s in streams}
    store = (nc.gpsimd, nc.scalar, nc.sync)[store_eng]

    import contextlib
    if pools is not None:
        cm_io, cm_mid, cm_op = (contextlib.nullcontext(p) for p in pools)
    else:
        cm_io = tc.tile_pool(name="io", bufs=in_bufs)
        cm_mid = tc.tile_pool(name="mid", bufs=bufs)
        cm_op = tc.tile_pool(name="op", bufs=out_bufs)
    with cm_io as io, cm_mid as mid, cm_op as op:
        starts = [sum(sizes[:i]) for i in range(len(sizes))]
        # store_delay: queue completed-tile stores and only issue each one
        # right before the loads `store_delay` tiles later, so the store's
        # scan-completion wait is already satisfied when the issuing engine's
        # sequencer reaches it (no in-order SEQ stall blocking later work)
        pending_stores = []

        def flush_stores(upto):
            while len(pending_stores) > upto:
                dst, src = pending_stores.pop(0)
                store.dma_start(out=dst, in_=src)

        tile_idx = -1
        for k, (k0, chunk_k) in enumerate(zip(starts, sizes)):
            for (b, d) in streams:
                tile_idx += 1
                # forward walks L ascending, backward descending
                l0 = k0 if d == 0 else L - k0 - chunk_k
                sl = slice(l0, l0 + chunk_k)

                sidx0 = 2 * b + d
                ld = {"sync": nc.sync, "scalar": nc.scalar,
                      "mix": (nc.sync, nc.scalar)[sidx0 % 2]}[load_eng]

                flush_stores(store_delay)

                g_t = io.tile([P, chunk_k], I8, tag="gin")
                h_t = io.tile([P, chunk_k], F16, tag="hin")
                gsrc = xg[b, d * P:(d + 1) * P, sl]
                hsrc = xh[b, d * P:(d + 1) * P, sl]
                # keep DMA granularity below the compute-chunk size: smaller
                # transfers pipeline better while compute runs full-width.
                # g lands first: it feeds the longer ACT->DVE chain.
                step = chunk_k // load_split
                for j in range(load_split):
                    js = slice(j * step, (j + 1) * step)
                    ld.dma_start(out=g_t[:, js], in_=gsrc[:, js])
                for j in range(load_split):
                    js = slice(j * step, (j + 1) * step)
                    ld.dma_start(out=h_t[:, js], in_=hsrc[:, js])

                sig = mid.tile([P, chunk_k], F16, tag="sig")
                nc.scalar.activation(sig, g_t, SIGMOID, scale=float(sg))
                if skip_coef:
                    coef = sig
                else:
                    coef = mid.tile([P, chunk_k], F16, tag="coef")
                # coef = sigmoid(-g) = 1 - sig: either a second ACT pass or a
                # DVE tensor_scalar (4x mode on f16). Both engines run near
                # the DMA roofline, so spread per-tile: coef_dve_mod routes
                # every Nth tile's coef to DVE (else per-stream coef_acts).
                # mod conventions: +N -> every Nth tile on the alternate
                # engine; -N -> every Nth tile on the primary engine
                if coef_dve_mod > 0:
                    on_act = tile_idx % coef_dve_mod != 0
                elif coef_dve_mod < 0:
                    on_act = tile_idx % -coef_dve_mod == 0
                else:
                    on_act = sidx0 < coef_acts
                if coef_act_k:
                    on_act = k in coef_act_k
                if skip_coef:
                    pass
                elif on_act:
                    nc.scalar.activation(coef, g_t, SIGMOID, scale=float(-sg))
                else:
                    nc.vector.tensor_scalar(
                        out=coef, in0=sig, scalar1=-1.0, scalar2=1.0,
                        op0=MULT, op1=ADD)
                if skip_mult:
                    v = sig
                else:
                    v = mid.tile([P, chunk_k], F16, tag="v")
                    # mult_pool(_mod): route some tiles' h*sig multiply to
                    # the Pool engine (0.42-efficient but otherwise idle) to
                    # shave the DVE below the DMA roofline
                    if mult_pool_k:
                        on_pool = k in mult_pool_k
                    elif mult_pool_mod > 0:
                        on_pool = tile_idx % mult_pool_mod == 0
                    elif mult_pool_mod < 0:
                        on_pool = tile_idx % -mult_pool_mod != 0
                    else:
                        on_pool = sidx0 < mult_pool
                    meng = nc.gpsimd if on_pool else nc.vector
                    meng.tensor_tensor(out=v, in0=h_t, in1=sig, op=MULT)

                out_t = op.tile([P, chunk_k], F16, tag="out")
                # scan_pool_mod: every Nth tile's scan runs on the Pool
                # engine instead of DVE -- slower per element but it unloads
                # the DVE (the serial scan is most of its busy time)
                if scan_pool_k:
                    seng = nc.gpsimd if k in scan_pool_k else nc.vector
                elif scan_pool_mod and tile_idx % scan_pool_mod == 0:
                    seng = nc.gpsimd
                else:
                    seng = nc.vector
                # scan_split: run the (inherently serial, 1 col/cycle) scan
                # in sub-pieces chained through carries, so each piece's
                # store fires while the next piece scans
                if skip_scan:
                    if skip_scan == 1:
                        nc.vector.tensor_tensor(
                            out=out_t, in0=coef, in1=v, op=MULT)
                        src_t = out_t
                    else:
                        src_t = v
                    if not skip_store:
                        pending_stores.append((
                            out[b, d * P:(d + 1) * P, sl], src_t))
                        if store_delay == 0:
                            flush_stores(0)
                    continue
                ssp = scan_split_last if k == len(sizes) - 1 else scan_split
                sstep = chunk_k // ssp
                for j in range(ssp):
                    jj = j if d == 0 else ssp - 1 - j
                    ssl = slice(jj * sstep, (jj + 1) * sstep)
                    init = carries[(b, d)]
                    if init is None:
                        init = 0.0
                    if d == 0:
                        seng.tensor_tensor_scan(
                            out=out_t[:, ssl], data0=coef[:, ssl],
                            data1=v[:, ssl], initial=init,
                            op0=MULT, op1=ADD)
                        carries[(b, d)] = out_t[:, ssl.stop - 1:ssl.stop]
                    else:
                        seng.tensor_tensor_scan(
                            out=out_t[:, ssl][:, ::-1],
                            data0=coef[:, ssl][:, ::-1],
                            data1=v[:, ssl][:, ::-1], initial=init,
                            op0=MULT, op1=ADD)
                        carries[(b, d)] = out_t[:, ssl.start:ssl.start + 1]

                    if skip_store:
                        continue
                    tstep = (ssl.stop - ssl.start) // store_split
                    for m in range(store_split):
                        ts = slice(ssl.start + m * tstep,
                                   ssl.start + (m + 1) * tstep)
                        pending_stores.append((
                            out[b, d * P:(d + 1) * P,
                                l0 + ts.start:l0 + ts.stop],
                            out_t[:, ts]))
                    if store_delay == 0:
                        flush_stores(0)
        flush_stores(0)


def _emit_pair(tc: tile.TileContext, xh, xg, out, sg=SG_DEFAULT, chunk=CHUNK,
               bufs=BUFS, out_bufs=4, store_eng=2, in_bufs=IN_BUFS, last=512,
               coef_act_k=(0, 1), load_split=1, scan_split=1, store_split=1,
               store_delay=2, store_pair=1, pools=None):
    """Pair-tile variant: both batches (B_PC=2) of a direction share one
    [128, 2, chunk] tile per tensor, halving DMA instruction count and
    running sig/coef/mult as single double-width instructions (the per-
    instruction SBUF-access overhead on ACT is ~185 ns, DVE ~60 ns).
    The serial scans stay per (batch, direction) on slices of the pair."""
    nc = tc.nc
    sizes = [chunk] * (L // chunk)
    if last:
        sizes = sizes[:-1] + [sizes[-1] - last, last]
    carries = {(b, d): None for b in range(B_PC) for d in (0, 1)}
    store = (nc.gpsimd, nc.scalar, nc.sync)[store_eng]

    import contextlib
    if pools is not None:
        cm_io, cm_mid, cm_op = (contextlib.nullcontext(p) for p in pools)
    else:
        cm_io = tc.tile_pool(name="io", bufs=in_bufs)
        cm_mid = tc.tile_pool(name="mid", bufs=bufs)
        cm_op = tc.tile_pool(name="op", bufs=out_bufs)
    with cm_io as io, cm_mid as mid, cm_op as op:
        starts = [sum(sizes[:i]) for i in range(len(sizes))]
        pending_stores = []

        def flush_stores(upto):
            while len(pending_stores) > upto:
                dst, src = pending_stores.pop(0)
                store.dma_start(out=dst, in_=src)

        for k, (k0, chunk_k) in enumerate(zip(starts, sizes)):
            for d in (0, 1):
                l0 = k0 if d == 0 else L - k0 - chunk_k
                sl = slice(l0, l0 + chunk_k)

                flush_stores(store_delay)

                g_t = io.tile([P, B_PC, chunk_k], I8, tag="gin")
                h_t = io.tile([P, B_PC, chunk_k], F16, tag="hin")
                gsrc = xg[:, d * P:(d + 1) * P, sl].rearrange("b p l -> p b l")
                hsrc = xh[:, d * P:(d + 1) * P, sl].rearrange("b p l -> p b l")
                step = chunk_k // load_split
                for j in range(load_split):
                    js = slice(j * step, (j + 1) * step)
                    nc.sync.dma_start(out=g_t[:, :, js], in_=gsrc[:, :, js])
                for j in range(load_split):
                    js = slice(j * step, (j + 1) * step)
                    nc.sync.dma_start(out=h_t[:, :, js], in_=hsrc[:, :, js])

                sig = mid.tile([P, B_PC, chunk_k], F16, tag="sig")
                nc.scalar.activation(sig, g_t, SIGMOID, scale=float(sg))
                coef = mid.tile([P, B_PC, chunk_k], F16, tag="coef")
                if k in coef_act_k:
                    nc.scalar.activation(coef, g_t, SIGMOID, scale=float(-sg))
                else:
                    nc.vector.tensor_scalar(
                        out=coef, in0=sig, scalar1=-1.0, scalar2=1.0,
                        op0=MULT, op1=ADD)
                v = mid.tile([P, B_PC, chunk_k], F16, tag="v")
                nc.vector.tensor_tensor(out=v, in0=h_t, in1=sig, op=MULT)

                out_t = op.tile([P, B_PC, chunk_k], F16, tag="out")
                sstep = chunk_k // scan_split
                for j in range(scan_split):
                    jj = j if d == 0 else scan_split - 1 - j
                    ssl = slice(jj * sstep, (jj + 1) * sstep)
                    for b in range(B_PC):
                        init = carries[(b, d)]
                        if init is None:
                            init = 0.0
                        ob = out_t[:, b, :]
                        cb = coef[:, b, :]
                        vb = v[:, b, :]
                        if d == 0:
                            nc.vector.tensor_tensor_scan(
                                out=ob[:, ssl], data0=cb[:, ssl],
                                data1=vb[:, ssl], initial=init,
                                op0=MULT, op1=ADD)
                            carries[(b, d)] = ob[:, ssl.stop - 1:ssl.stop]
                        else:
                            nc.vector.tensor_tensor_scan(
                                out=ob[:, ssl][:, ::-1],
                                data0=cb[:, ssl][:, ::-1],
                                data1=vb[:, ssl][:, ::-1], initial=init,
                                op0=MULT, op1=ADD)
                            carries[(b, d)] = ob[:, ssl.start:ssl.start + 1]

                dst = out[:, d * P:(d + 1) * P, sl].rearrange("b p l -> p b l")
                if store_pair:
                    tstep = chunk_k // store_split
                    for m in range(store_split):
                        ts = slice(m * tstep, (m + 1) * tstep)
                        pending_stores.append((dst[:, :, ts], out_t[:, :, ts]))
                else:
                    for b in range(B_PC):
                        pending_stores.append((dst[:, b, :], out_t[:, b, :]))
                if store_delay == 0:
                    flush_stores(0)
        flush_stores(0)


_NC_CACHE = {}


def build(n_repeat=1, share_pools=True, pair=False, **emit_kwargs):
    key = (n_repeat, share_pools, pair, tuple(sorted(emit_kwargs.items())))
    if key not in _NC_CACHE:
        emit = _emit_pair if pair else _emit
        nc = bacc.Bacc("TRN2", target_bir_lowering=False, debug=False)
        xh = nc.dram_tensor("xh", [B_PC, H // 2, L], F16, kind="ExternalInput")
        xg = nc.dram_tensor("xg", [B_PC, H // 2, L], I8, kind="ExternalInput")
        out = nc.dram_tensor("out", [B_PC, H // 2, L], F16, kind="ExternalOutput")
        with tile.TileContext(nc) as tc:
            if share_pools and n_repeat > 1:
                dflt_ob = 4 if pair else OUT_BUFS
                with tc.tile_pool(name="io", bufs=emit_kwargs.get("in_bufs", IN_BUFS)) as io, \
                     tc.tile_pool(name="mid", bufs=emit_kwargs.get("bufs", BUFS)) as mid, \
                     tc.tile_pool(name="op", bufs=emit_kwargs.get("out_bufs", dflt_ob)) as op:
                    for _ in range(n_repeat):
                        emit(tc, xh.ap(), xg.ap(), out.ap(),
                             pools=(io, mid, op), **emit_kwargs)
            else:
                for _ in range(n_repeat):
                    emit(tc, xh.ap(), xg.ap(), out.ap(), **emit_kwargs)
        nc.compile()
        _NC_CACHE[key] = nc
    return _NC_CACHE[key]


def prepare_inputs(x: np.ndarray):
    """Split/quantize the full x into per-kernel input tensors (host side).

    Returns (in_arrays, sg): dict of full-shape arrays whose axis 0 shards
    across cores, and the g dequant scale baked into the program.
    """
    hsel = np.r_[0:P, 2 * P:3 * P]          # forward h, backward h channels
    gsel = np.r_[P:2 * P, 3 * P:4 * P]      # forward g, backward g channels
    xh = np.ascontiguousarray(x[:, hsel, :]).astype(np.float16)
    g = x[:, gsel, :]
    sg = float(np.abs(g).max()) / 127.0
    sg = float(np.float32(sg))
    xg = np.clip(np.rint(g * (1.0 / sg)), -127, 127).astype(np.int8)
    return {"xh": xh, "xg": xg}, sg


def kernel(x: np.ndarray):
    assert x.shape == (B, H, L) and x.dtype == np.float32
    ins, sg = prepare_inputs(x)
    nc = build(sg=sg)
    in_maps = [
        {k: np.ascontiguousarray(v[i * B_PC:(i + 1) * B_PC])
         for k, v in ins.items()}
        for i in range(N_CORES)
    ]
    res = run_bass_kernel_spmd(nc, in_maps, core_ids=list(range(N_CORES)))
    return np.concatenate(
        [r["out"] for r in res.results], axis=0).astype(np.float32)
